# revision 1
# baseline (speedup 1.0000x reference)
"""Policy-network kernel for Trainium2 (Bass/Tile), SPMD over 8 NeuronCores.

Strategy: data-parallel over batch B=128 -> 16 batches per core; all tables
and MLP weights replicated; no collectives. Heavy matmuls run as float32r
(full PE rate at N>=256). The per-(b,a) relation gather of the attention
output is fused on-chip via a one-hot matmul; rel/ent embedding row gathers
use indirect DMA.
"""

import numpy as np

# Problem dims (hardcoded per contract)
B, S, Dw, Dr, De, H, R, E, A = 128, 32, 300, 256, 256, 512, 512, 50000, 256
ACT = Dr + De          # 512
NCORES = 8
BL = B // NCORES       # 16 batches per core
BSL = BL * S           # 512 rows per core
NEG = -1e9

_CACHE = {}


def _build():
    import concourse.bass as bass
    import concourse.tile as tile
    from concourse import bacc, mybir

    f32 = mybir.dt.float32
    f32r = mybir.dt.float32r
    i32 = mybir.dt.int32
    ts = bass.ts

    nc = bacc.Bacc("TRN2", target_bir_lowering=False, debug=False)

    def din(name, shape, dt=f32):
        return nc.dram_tensor(name, shape, dt, kind="ExternalInput").ap()

    xT = din("xT", [Dw, BSL], f32r)              # transposed transformer_output shard
    w_step = din("w_step", [Dw, Dr], f32r)
    b_step = din("b_step", [Dr, 1])
    relwT = din("relwT", [Dr, R], f32r)          # (rel_emb * w_att).T
    mask_row = din("mask_row", [1, BSL], f32r)   # 0 / NEG per (b,s)
    iota4 = din("iota4", [128, 4])         # iota4[p,t] = t*128+p
    ridx_f = din("ridx_f", [BL, A])        # r_space as f32
    gidx_r = nc.dram_tensor("gidx_r", [128, 2 * BL], i32, kind="ExternalInput").ap()
    gidx_e = nc.dram_tensor("gidx_e", [128, 2 * BL], i32, kind="ExternalInput").ap()
    rel_emb = din("rel_emb", [R, Dr])
    ent_emb = din("ent_emb", [E, De])
    phT = din("phT", [H, BL], f32r)              # path_hidden.T shard
    w1a = din("w1a", [H, ACT], f32r)             # W1[:H]
    w1b = din("w1b", [Dr, ACT], f32r)            # W1[H:]
    b1 = din("b1", [ACT, 1])
    w2 = din("w2", [ACT, ACT], f32r)
    amask = din("amask", [BL, A])          # 0 / NEG
    ones_row = din("ones_row", [1, 128], f32r)

    out_dram = nc.dram_tensor("out", [BL, A], f32, kind="ExternalOutput").ap()
    logits_dram = [nc.dram_tensor(f"logits_scratch{h}", [BL // 2, A], f32).ap()
                   for h in range(2)]

    KD = [(0, 128), (128, 256), (256, 300)]   # Dw K-tiles

    from concourse.masks import make_identity

    with tile.TileContext(nc) as tc:
        with (
            tc.tile_pool(name="const", bufs=1) as cpool,
            tc.tile_pool(name="work", bufs=2) as wpool,
            tc.tile_pool(name="perb", bufs=4) as bpool,
            tc.tile_pool(name="ps_big", bufs=2, space="PSUM") as ps_big,
            tc.tile_pool(name="ps_h2", bufs=2, space="PSUM") as ps_h2p,
            tc.tile_pool(name="ps_med", bufs=2, space="PSUM") as ps_med,
            tc.tile_pool(name="ps_small", bufs=2, space="PSUM") as ps_small,
        ):
            # ---- constants to SBUF ----
            ident = cpool.tile([128, 128], f32, tag="ident")
            make_identity(nc, ident[:])
            ones1 = cpool.tile([1, 128], f32r, tag="ones1")
            nc.sync.dma_start(ones1[:], ones_row[:])

            def load_const(tag, src, p0, p1, cols, dt=f32r):
                t = cpool.tile([p1 - p0, cols], dt, tag=tag)
                nc.sync.dma_start(t[:], src[p0:p1, :])
                return t
            b_step_sb = [load_const(f"bs{k}", b_step, k * 128, (k + 1) * 128, 1, f32) for k in range(2)]
            b1_sb = [load_const(f"b1{k}", b1, k * 128, (k + 1) * 128, 1, f32) for k in range(4)]
            mask_sb = cpool.tile([1, BSL], f32r, tag="mask")
            nc.sync.dma_start(mask_sb[:], mask_row[:])
            iota_sb = cpool.tile([128, 4], f32, tag="iota")
            nc.sync.dma_start(iota_sb[:], iota4[:])
            # warm the ACT function table immediately (no data deps beyond iota)
            act_warm = cpool.tile([128, 4], f32, tag="act_warm")
            nc.scalar.activation(act_warm[:], iota_sb[:],
                                 bass.mybir.ActivationFunctionType.Exp, scale=0.0)
            w_step_sb = [load_const(f"ws{k}", w_step, a, b_, Dr) for k, (a, b_) in enumerate(KD)]
            xT_sb = [load_const(f"xt{k}", xT, a, b_, BSL) for k, (a, b_) in enumerate(KD)]
            relwT_sb = [load_const(f"rw{k}", relwT, k * 128, (k + 1) * 128, R) for k in range(2)]
            # ---- hoisted embedding-row gathers: issued up-front, overlap compute ----
            gidx_r_sb = cpool.tile([128, 2 * BL], i32, tag="gidx_r")
            nc.sync.dma_start(gidx_r_sb[:], gidx_r[:])
            gidx_e_sb = cpool.tile([128, 2 * BL], i32, tag="gidx_e")
            nc.sync.dma_start(gidx_e_sb[:], gidx_e[:])
            g_all = cpool.tile([128, 2 * BL, ACT], f32, tag="g_all")
            for j in range(2 * BL):
                nc.gpsimd.indirect_dma_start(
                    out=g_all[:, j, 0:Dr], out_offset=None, in_=rel_emb[:],
                    in_offset=bass.IndirectOffsetOnAxis(ap=gidx_r_sb[:, j:j + 1], axis=0))
                nc.gpsimd.indirect_dma_start(
                    out=g_all[:, j, Dr:ACT], out_offset=None, in_=ent_emb[:],
                    in_offset=bass.IndirectOffsetOnAxis(ap=gidx_e_sb[:, j:j + 1], axis=0))

            w1b_sb = [load_const(f"w1b{k}", w1b, k * 128, (k + 1) * 128, ACT) for k in range(2)]
            w1a_sb = [load_const(f"w1a{k}", w1a, k * 128, (k + 1) * 128, ACT) for k in range(4)]
            phT_sb = [load_const(f"ph{k}", phT, k * 128, (k + 1) * 128, BL) for k in range(4)]
            w2_sb = [load_const(f"w2{k}", w2, k * 128, (k + 1) * 128, ACT) for k in range(4)]
            amask_h = []
            for h in range(2):
                am = cpool.tile([BL // 2, A], f32, tag=f"amask{h}")
                nc.sync.dma_start(am[:], amask[h * (BL // 2):(h + 1) * (BL // 2), :])
                amask_h.append(am)

            def r(ap):
                return ap


            # ---- saqT = tanh(W_step.T @ xT + b_step)  [2][128, BSL] ----
            saqT_sb = []
            for t in range(2):
                ps = ps_big.tile([128, BSL], f32, tag="big")
                for k in range(3):
                    nc.tensor.matmul(ps[:], r(w_step_sb[k][:, ts(t, 128)]), r(xT_sb[k][:]),
                                     start=(k == 0), stop=(k == 2))
                sb = cpool.tile([128, BSL], f32r, tag=f"saqT{t}")
                nc.scalar.activation(sb[:], ps[:], bass.mybir.ActivationFunctionType.Tanh,
                                     bias=b_step_sb[t][:])
                saqT_sb.append(sb)

            # ---- scores + masked softmax per r-tile -> alpha [4][128, BL, S] ----
            alpha_sb = []
            for rt in range(4):
                ps = ps_big.tile([128, BSL], f32, tag="big")
                for k in range(2):
                    nc.tensor.matmul(ps[:], r(relwT_sb[k][:, ts(rt, 128)]), r(saqT_sb[k][:]),
                                     start=(k == 0), stop=False)
                nc.tensor.matmul(ps[:], r(ones1[:]), r(mask_sb[:]), start=False, stop=True)
                al = cpool.tile([128, BL, S], f32r, tag=f"alpha{rt}")
                nc.scalar.activation(al[:].rearrange("p b s -> p (b s)"), ps[:],
                                     bass.mybir.ActivationFunctionType.Exp)
                sums = wpool.tile([128, BL], f32, tag="sums")
                nc.vector.tensor_reduce(sums[:], al[:], axis=bass.mybir.AxisListType.X,
                                        op=bass.mybir.AluOpType.add)
                rec = wpool.tile([128, BL], f32, tag="rec")
                nc.vector.reciprocal(rec[:], sums[:])
                nc.vector.tensor_mul(al[:], al[:],
                                     rec[:].unsqueeze(2).to_broadcast((128, BL, S)))
                alpha_sb.append(al)

            # ---- saq natural per b: [32, BL, Dr] via PE transpose ----
            saq_nat = cpool.tile([32, BL, Dr], f32r, tag="saq_nat")
            for b in range(BL):
                ps = ps_small.tile([32, Dr], f32, tag="small")
                for t in range(2):
                    nc.tensor.transpose(ps[:, ts(t, 128)],
                                        saqT_sb[t][:, b * S:(b + 1) * S].bitcast(f32),
                                        ident[:])
                nc.scalar.copy(saq_nat[:, b, :], ps[:])

            # ---- ph contribution + fused h1 bias: biasT[t][:,b] = (ph@W1a)T + b1 ----
            biasT = cpool.tile([128, 4, BL], f32, tag="biasT")
            for t in range(4):
                ps = ps_med.tile([128, BL], f32, tag="med")
                for k in range(4):
                    nc.tensor.matmul(ps[:], r(w1a_sb[k][:, ts(t, 128)]), r(phT_sb[k][:]),
                                     start=(k == 0), stop=(k == 3))
                nc.vector.tensor_scalar_add(biasT[:, t, :], ps[:], b1_sb[t][:])

            # ---- per-batch pipeline, stage-major over groups of 4 ----
            onehots, alpha_gTs, raq_gTs, h1Ts = {}, {}, {}, {}
            for g in range(BL // 4):
                bs = range(4 * g, 4 * g + 4)
                for b in bs:
                    idxb = bpool.tile([128, A], f32, tag="idxb", bufs=3)
                    nc.sync.dma_start(idxb[:], ridx_f[b:b + 1, :].partition_broadcast(128))
                    onehot = bpool.tile([128, 4, A], f32r, tag="onehot", bufs=5)
                    nc.vector.tensor_tensor(
                        onehot[:],
                        idxb[:].unsqueeze(1).to_broadcast((128, 4, A)),
                        iota_sb[:].unsqueeze(2).to_broadcast((128, 4, A)),
                        op=bass.mybir.AluOpType.is_equal)
                    onehots[b] = onehot
                for b in bs:
                    ps_ag = ps_small.tile([S, A], f32, tag="small")
                    for rt in range(4):
                        nc.tensor.matmul(ps_ag[:], r(alpha_sb[rt][:, b, :]),
                                         r(onehots[b][:, rt, :]),
                                         start=(rt == 0), stop=(rt == 3))
                    alpha_gT = bpool.tile([S, A], f32r, tag="alpha_gT", bufs=6)
                    nc.vector.tensor_copy(alpha_gT[:], ps_ag[:])
                    alpha_gTs[b] = alpha_gT
                for b in bs:
                    raq_gT = bpool.tile([128, 2, A], f32r, tag="raq_gT", bufs=5)
                    ps_rq = ps_big.tile([128, 2 * A], f32, tag="big")
                    for dt_ in range(2):
                        nc.tensor.matmul(ps_rq[:, ts(dt_, A)], r(saq_nat[:, b, ts(dt_, 128)]),
                                         r(alpha_gTs[b][:]), start=True, stop=True)
                    nc.vector.tensor_copy(raq_gT[:], ps_rq[:].rearrange("p (d a) -> p d a", a=A))
                    raq_gTs[b] = raq_gT
                for b in bs:
                    h1T = bpool.tile([128, 4, A], f32r, tag="h1T", bufs=5)
                    for t in range(4):
                        ps_h1 = ps_med.tile([128, A], f32, tag="med")
                        for k in range(2):
                            nc.tensor.matmul(ps_h1[:], r(w1b_sb[k][:, ts(t, 128)]),
                                             r(raq_gTs[b][:, k, :]), start=(k == 0), stop=(k == 1))
                        nc.scalar.activation(h1T[:, t, :], ps_h1[:],
                                             bass.mybir.ActivationFunctionType.Relu,
                                             bias=biasT[:, t, b:b + 1])
                    h1Ts[b] = h1T
                for b in bs:
                    for c in range(2):
                        ps_h2 = ps_h2p.tile([128, ACT], f32, tag="h2")
                        for k in range(4):
                            nc.tensor.matmul(ps_h2[:], r(h1Ts[b][:, k, ts(c, 128)]), r(w2_sb[k][:]),
                                             start=(k == 0), stop=(k == 3))
                        scratch = bpool.tile([128, ACT], f32, tag="scratch", bufs=3)
                        nc.vector.tensor_mul(scratch[:], ps_h2[:], g_all[:, b * 2 + c, :])
                        trash = bpool.tile([128, ACT], f32, tag="trash", bufs=2)
                        lg = bpool.tile([128, 1], f32, tag="lg")
                        nc.scalar.activation(trash[:], scratch[:],
                                             bass.mybir.ActivationFunctionType.Identity,
                                             accum_out=lg[:])
                        nc.sync.dma_start(
                            logits_dram[b // (BL // 2)][b % (BL // 2), ts(c, 128)]
                            .rearrange("(p o) -> p o", o=1), lg[:])

            # ---- final action softmax [BL, A], two halves for tail overlap ----
            HB = BL // 2
            for h in range(2):
                sl = slice(h * HB, (h + 1) * HB)
                lsb = wpool.tile([HB, A], f32, tag="lsb")
                nc.sync.dma_start(lsb[:], logits_dram[h][:])
                nc.vector.tensor_add(lsb[:], lsb[:], amask_h[h][:])
                negmaxF = wpool.tile([HB, 1], f32, tag="negmaxF")
                nc.vector.tensor_reduce(negmaxF[:], lsb[:], axis=bass.mybir.AxisListType.X,
                                        op=bass.mybir.AluOpType.max, negate=True)
                expF = wpool.tile([HB, A], f32, tag="expF")
                sumsF = wpool.tile([HB, 1], f32, tag="sumsF")
                nc.scalar.activation(expF[:], lsb[:], bass.mybir.ActivationFunctionType.Exp,
                                     bias=negmaxF[:], accum_out=sumsF[:])
                recF = wpool.tile([HB, 1], f32, tag="recF")
                nc.vector.reciprocal(recF[:], sumsF[:])
                osb = wpool.tile([HB, A], f32, tag="osb")
                nc.vector.tensor_scalar_mul(osb[:], expF[:], recF[:])
                nc.sync.dma_start(out_dram[sl, :], osb[:])

    nc.compile()
    return nc


def _host_prep(inputs):
    """Build the 8 per-core input maps from full inputs."""
    x = np.asarray(inputs["transformer_output"], np.float32)
    qmask = np.asarray(inputs["question_mask"])
    W_step = np.ascontiguousarray(np.asarray(inputs["W_step"], np.float32))
    b_step = np.asarray(inputs["b_step"], np.float32).reshape(Dr, 1)
    w_att = np.asarray(inputs["w_att"], np.float32)
    rel_emb = np.ascontiguousarray(np.asarray(inputs["rel_emb"], np.float32))
    ent_emb = np.ascontiguousarray(np.asarray(inputs["ent_emb"], np.float32))
    ph = np.asarray(inputs["path_hidden"], np.float32)
    W1 = np.asarray(inputs["W1"], np.float32)
    b1 = np.asarray(inputs["b1"], np.float32).reshape(ACT, 1)
    W2 = np.ascontiguousarray(np.asarray(inputs["W2"], np.float32))
    b2 = np.asarray(inputs["b2"], np.float32).reshape(1, ACT)
    r_space = np.asarray(inputs["r_space"], np.int32)
    e_space = np.asarray(inputs["e_space"], np.int32)
    action_mask = np.asarray(inputs["action_mask"], np.float32)

    relwT = np.ascontiguousarray((rel_emb * w_att[None, :]).T)   # [Dr, R]
    w1a = np.ascontiguousarray(W1[:H])
    w1b = np.ascontiguousarray(W1[H:])
    iota4 = (np.arange(128, dtype=np.float32)[:, None]
             + 128.0 * np.arange(4, dtype=np.float32)[None, :])
    iota4 = np.ascontiguousarray(iota4)

    in_maps = []
    for i in range(NCORES):
        b0, b1_ = i * BL, (i + 1) * BL
        xs = x[b0:b1_].reshape(BSL, Dw)
        mrow = np.where(qmask[b0:b1_].reshape(1, BSL), np.float32(NEG), np.float32(0.0))
        amask_add = np.where(action_mask[b0:b1_] > 0, np.float32(0.0), np.float32(NEG))
        c_rel = rel_emb @ b2[0, :Dr]
        c_ent = ent_emb @ b2[0, Dr:]
        amask_add = amask_add + c_rel[r_space[b0:b1_]] + c_ent[e_space[b0:b1_]]
        in_maps.append({
            "xT": np.ascontiguousarray(xs.T),
            "w_step": W_step,
            "b_step": b_step,
            "relwT": relwT,
            "mask_row": np.ascontiguousarray(mrow),
            "iota4": iota4,
            "ridx_f": np.ascontiguousarray(r_space[b0:b1_].astype(np.float32)),
            "gidx_r": np.ascontiguousarray(
                r_space[b0:b1_].reshape(BL, 2, 128).transpose(2, 0, 1).reshape(128, 2 * BL)),
            "gidx_e": np.ascontiguousarray(
                e_space[b0:b1_].reshape(BL, 2, 128).transpose(2, 0, 1).reshape(128, 2 * BL)),
            "rel_emb": rel_emb,
            "ent_emb": ent_emb,
            "phT": np.ascontiguousarray(ph[b0:b1_].T),
            "w1a": w1a,
            "w1b": w1b,
            "b1": b1,
            "w2": W2,
            "amask": np.ascontiguousarray(amask_add),
            "ones_row": np.ones((1, 128), np.float32),
        })
    return in_maps


def kernel(**inputs):
    from concourse.bass_utils import run_bass_kernel_spmd

    if "nc" not in _CACHE:
        _CACHE["nc"] = _build()
    nc = _CACHE["nc"]
    in_maps = _host_prep(inputs)
    res = run_bass_kernel_spmd(nc, in_maps, list(range(NCORES)))
    return np.concatenate([res.results[i]["out"] for i in range(NCORES)], axis=0)



# revision 25
# speedup vs baseline: 1.2362x; 1.2362x over previous
"""Policy-network kernel for Trainium2 (Bass/Tile), SPMD over 8 NeuronCores.

Strategy: data-parallel over batch B=128 -> 16 batches per core; MLP weights
replicated; no collectives. Action embedding rows (rel_emb[r_space],
ent_emb[e_space]) are gathered host-side during input prep and shipped as one
bf16 tensor, so the device does no indirect DMA. Matmul operands are bf16
(same PE rate as float32r at N>=256, half the SBUF/DMA traffic). The per-(b,a)
relation gather of the attention output is fused on-chip via a one-hot matmul
whose one-hots are pre-scaled by the softmax normalizer. Final logits stay in
SBUF and are transposed on the PE for the action softmax (no DRAM round-trip).
"""

import numpy as np

# Problem dims (hardcoded per contract)
B, S, Dw, Dr, De, H, R, E, A = 128, 32, 300, 256, 256, 512, 512, 50000, 256
ACT = Dr + De          # 512
NCORES = 8
BL = B // NCORES       # 16 batches per core
BSL = BL * S           # 512 rows per core
NEG = -1e9

_CACHE = {}


def _build():
    import concourse.bass as bass
    import concourse.tile as tile
    from concourse import bacc, mybir

    f32 = mybir.dt.float32
    bf16 = mybir.dt.bfloat16
    ts = bass.ts

    nc = bacc.Bacc("TRN2", target_bir_lowering=False, debug=False)

    def din(name, shape, dt=bf16):
        return nc.dram_tensor(name, shape, dt, kind="ExternalInput").ap()

    xT = din("xT", [Dw, BSL])                    # transposed transformer_output shard
    w_step = din("w_step", [Dw, Dr])
    b_step = din("b_step", [Dr, 1], f32)
    b_step_row = din("b_step_row", [1, Dr])      # b_step as bf16 row for free-dim bias
    relwT = din("relwT", [Dr, R])                # (rel_emb * w_att).T
    mask_row = din("mask_row", [1, BSL])         # 0 / NEG per (b,s)
    oh_d = din("oh", [128, BL, 4, A])            # host-built one-hots of r_space
    g_all_d = din("g_all", [128, 2 * BL, ACT])   # host-gathered action embeddings
    phT = din("phT", [H, BL])                    # path_hidden.T shard
    w1a = din("w1a", [H, ACT])                   # W1[:H]
    w1b = din("w1b", [Dr, ACT])                  # W1[H:]
    b1 = din("b1", [ACT, 1], f32)
    w2 = din("w2", [ACT, ACT])
    amask = din("amask", [BL, A], f32)           # 0 / NEG with folded b2 terms
    ones_row = din("ones_row", [1, 128])

    out_dram = nc.dram_tensor("out", [BL, A], f32, kind="ExternalOutput").ap()

    KD = [(0, 128), (128, 256), (256, 300)]   # Dw K-tiles

    from concourse.masks import make_identity

    with tile.TileContext(nc) as tc:
        with (
            tc.tile_pool(name="const", bufs=1) as cpool,
            tc.tile_pool(name="work", bufs=2) as wpool,
            tc.tile_pool(name="perb", bufs=4) as bpool,
            tc.tile_pool(name="ps_big", bufs=2, space="PSUM") as ps_big,
            tc.tile_pool(name="ps_h2", bufs=2, space="PSUM") as ps_h2p,
            tc.tile_pool(name="ps_med", bufs=3, space="PSUM") as ps_med,
        ):
            # ---- constants to SBUF ----
            ident_f32 = cpool.tile([128, 128], f32, tag="ident_f32")
            make_identity(nc, ident_f32[:])
            ones1 = cpool.tile([1, 128], bf16, tag="ones1")
            nc.sync.dma_start(ones1[:], ones_row[:])

            def load_const(tag, src, p0, p1, cols, dt=bf16):
                t = cpool.tile([p1 - p0, cols], dt, tag=tag)
                nc.sync.dma_start(t[:], src[p0:p1, :])
                return t
            # order: tensors gating the first computations go first
            w_step_sb = [load_const(f"ws{k}", w_step, a, b_, Dr) for k, (a, b_) in enumerate(KD)]
            xT_sb = [load_const(f"xt{k}", xT, a, b_, BSL) for k, (a, b_) in enumerate(KD)]
            b_step_sb = [load_const(f"bs{k}", b_step, k * 128, (k + 1) * 128, 1, f32) for k in range(2)]
            b_step_row_sb = cpool.tile([1, Dr], bf16, tag="b_step_row")
            nc.sync.dma_start(b_step_row_sb[:], b_step_row[:])
            relwT_sb = [load_const(f"rw{k}", relwT, k * 128, (k + 1) * 128, R) for k in range(2)]
            mask_sb = cpool.tile([1, BSL], bf16, tag="mask")
            nc.sync.dma_start(mask_sb[:], mask_row[:])
            # warm the ACT function table immediately (no data deps beyond b_step)
            act_warm = cpool.tile([128, 1], f32, tag="act_warm")
            nc.scalar.activation(act_warm[:], b_step_sb[0][:],
                                 bass.mybir.ActivationFunctionType.Exp, scale=0.0)
            oh_sb = cpool.tile([128, BL, 4, A], bf16, tag="oh_sb")
            for q in range(4):
                nc.sync.dma_start(oh_sb[:, ts(q, BL // 4), :, :],
                                  oh_d[:, ts(q, BL // 4), :, :])
            b1_sb = [load_const(f"b1{k}", b1, k * 128, (k + 1) * 128, 1, f32) for k in range(4)]
            w1a_sb = [load_const(f"w1a{k}", w1a, k * 128, (k + 1) * 128, ACT) for k in range(4)]
            phT_sb = [load_const(f"ph{k}", phT, k * 128, (k + 1) * 128, BL) for k in range(4)]
            w1b_sb = [load_const(f"w1b{k}", w1b, k * 128, (k + 1) * 128, ACT) for k in range(2)]
            w2_sb = [load_const(f"w2{k}", w2, k * 128, (k + 1) * 128, ACT) for k in range(4)]
            # host-gathered action embeddings: 4 column-chunk DMAs to spread queues
            g_all = cpool.tile([128, 2 * BL, ACT], bf16, tag="g_all")
            for q in range(4):
                nc.sync.dma_start(g_all[:, ts(q, BL // 2), :],
                                  g_all_d[:, ts(q, BL // 2), :])
            amask_sb = cpool.tile([BL, A], f32, tag="amask")
            nc.sync.dma_start(amask_sb[:], amask[:])

            # ---- saqT = tanh(W_step.T @ xT + b_step)  [2][128, BSL] bf16 ----
            saqT_sb = []
            for t in range(2):
                ps = ps_big.tile([128, BSL], f32, tag="big")
                for k in range(3):
                    nc.tensor.matmul(ps[:], w_step_sb[k][:, ts(t, 128)], xT_sb[k][:],
                                     start=(k == 0), stop=(k == 2))
                sb = cpool.tile([128, BSL], bf16, tag=f"saqT{t}")
                nc.scalar.activation(sb[:], ps[:], bass.mybir.ActivationFunctionType.Tanh,
                                     bias=b_step_sb[t][:])
                saqT_sb.append(sb)

            # ---- saq in natural row layout [128, 4, Dr]: chunk q holds rows
            # (b,s) = q*128+p, i.e. batches 4q..4q+3 (32 seq rows each) ----
            saq_nat = cpool.tile([128, 4, Dr], bf16, tag="saq_nat")
            for q in range(4):
                ps = ps_med.tile([128, Dr], f32, tag="med")
                for k in range(3):
                    nc.tensor.matmul(ps[:], xT_sb[k][:, ts(q, 128)], w_step_sb[k][:],
                                     start=(k == 0), stop=False)
                nc.tensor.matmul(ps[:], ones1[:], b_step_row_sb[:],
                                 start=False, stop=True)
                nc.scalar.activation(saq_nat[:, q, :], ps[:],
                                     bass.mybir.ActivationFunctionType.Tanh)

            # ---- scores -> normalized alpha [4][128, BL, S] bf16 (softmax over S;
            # exp on scalar, 1/sum applied as a broadcast mult on gpsimd) ----
            alpha_sb = []
            for rt in range(4):
                ps = ps_big.tile([128, BSL], f32, tag="big")
                for k in range(2):
                    nc.tensor.matmul(ps[:], relwT_sb[k][:, ts(rt, 128)], saqT_sb[k][:],
                                     start=(k == 0), stop=False)
                nc.tensor.matmul(ps[:], ones1[:], mask_sb[:], start=False, stop=True)
                al = wpool.tile([128, BL, S], bf16, tag="al_raw")
                nc.scalar.activation(al[:].rearrange("p b s -> p (b s)"), ps[:],
                                     bass.mybir.ActivationFunctionType.Exp)
                sums = wpool.tile([128, BL], f32, tag="sums")
                nc.vector.tensor_reduce(sums[:], al[:], axis=bass.mybir.AxisListType.X,
                                        op=bass.mybir.AluOpType.add)
                rec = wpool.tile([128, BL], f32, tag="rec")
                nc.vector.reciprocal(rec[:], sums[:])
                al_sc = cpool.tile([128, BL, S], bf16, tag=f"alpha{rt}")
                nc.gpsimd.tensor_tensor(al_sc[:], al[:],
                                        rec[:].unsqueeze(2).to_broadcast((128, BL, S)),
                                        op=bass.mybir.AluOpType.mult)
                alpha_sb.append(al_sc)

            # ---- ph contribution + fused h1 bias: biasT[t][:,b] = (ph@W1a)T + b1 ----
            biasT = cpool.tile([128, 4, BL], f32, tag="biasT")
            for t in range(4):
                ps = ps_med.tile([128, BL], f32, tag="med")
                for k in range(4):
                    nc.tensor.matmul(ps[:], w1a_sb[k][:, ts(t, 128)], phT_sb[k][:],
                                     start=(k == 0), stop=(k == 3))
                nc.vector.tensor_scalar_add(biasT[:, t, :], ps[:], b1_sb[t][:])

            # logits accumulators: lg_c[c][:, b] = logits for actions c*128..c*128+127
            lg_c = []
            for c in range(2):
                lg = cpool.tile([128, BL], f32, tag=f"lg{c}")
                lg_c.append(lg)

            # ---- per-batch pipeline, stage-major over groups of 4 ----
            alpha_gTs, raq_gTs, h1Ts = {}, {}, {}
            for g in range(BL // 4):
                bs = range(4 * g, 4 * g + 4)
                for b in bs:
                    off = (b % 4) * S
                    ps_ag = ps_med.tile([128, A], f32, tag="med")
                    for rt in range(4):
                        nc.tensor.matmul(ps_ag[off:off + S, :], alpha_sb[rt][:, b, :],
                                         oh_sb[:, b, rt, :],
                                         start=(rt == 0), stop=(rt == 3),
                                         tile_position=(0, off))
                    alpha_gT = bpool.tile([128, A], bf16, tag="alpha_gT", bufs=6)
                    nc.vector.tensor_copy(alpha_gT[off:off + S, :], ps_ag[off:off + S, :])
                    alpha_gTs[b] = alpha_gT
                for b in bs:
                    off = (b % 4) * S
                    raq_gT = bpool.tile([128, 2, A], bf16, tag="raq_gT", bufs=5)
                    ps_rq = ps_big.tile([128, 2 * A], f32, tag="big")
                    sq = saq_nat[off:off + S, b // 4, :]
                    for dt_ in range(2):
                        nc.tensor.matmul(ps_rq[:, ts(dt_, A)], sq[:, ts(dt_, 128)],
                                         alpha_gTs[b][off:off + S, :], start=True, stop=True,
                                         tile_position=(off, 0))
                    nc.scalar.copy(raq_gT[:], ps_rq[:].rearrange("p (d a) -> p d a", a=A))
                    raq_gTs[b] = raq_gT
                for b in bs:
                    h1T = bpool.tile([128, 4, A], bf16, tag="h1T", bufs=5)
                    for t in range(4):
                        ps_h1 = ps_med.tile([128, A], f32, tag="med")
                        for k in range(2):
                            nc.tensor.matmul(ps_h1[:], w1b_sb[k][:, ts(t, 128)],
                                             raq_gTs[b][:, k, :], start=(k == 0), stop=(k == 1))
                        nc.scalar.activation(h1T[:, t, :], ps_h1[:],
                                             bass.mybir.ActivationFunctionType.Relu,
                                             bias=biasT[:, t, b:b + 1])
                    h1Ts[b] = h1T
                for b in bs:
                    for c in range(2):
                        ps_h2 = ps_h2p.tile([128, ACT], f32, tag="h2")
                        for k in range(4):
                            nc.tensor.matmul(ps_h2[:], h1Ts[b][:, k, ts(c, 128)], w2_sb[k][:],
                                             start=(k == 0), stop=(k == 3))
                        scratch = bpool.tile([128, ACT], f32, tag="scratch", bufs=3)
                        nc.vector.tensor_mul(scratch[:], ps_h2[:], g_all[:, b * 2 + c, :])
                        nc.vector.tensor_reduce(lg_c[c][:, b:b + 1], scratch[:],
                                                axis=bass.mybir.AxisListType.X,
                                                op=bass.mybir.AluOpType.add)

            # ---- tail: transpose logits on PE, masked softmax, single out DMA ----
            ps_t = ps_med.tile([BL, A], f32, tag="med")
            for c in range(2):
                nc.tensor.transpose(ps_t[:, ts(c, 128)], lg_c[c][:], ident_f32[:])
            lsb = wpool.tile([BL, A], f32, tag="lsb")
            nc.vector.tensor_add(lsb[:], ps_t[:], amask_sb[:])
            negmaxF = wpool.tile([BL, 1], f32, tag="negmaxF")
            nc.vector.tensor_reduce(negmaxF[:], lsb[:], axis=bass.mybir.AxisListType.X,
                                    op=bass.mybir.AluOpType.max, negate=True)
            expF = wpool.tile([BL, A], f32, tag="expF")
            sumsF = wpool.tile([BL, 1], f32, tag="sumsF")
            nc.scalar.activation(expF[:], lsb[:], bass.mybir.ActivationFunctionType.Exp,
                                 bias=negmaxF[:], accum_out=sumsF[:])
            recF = wpool.tile([BL, 1], f32, tag="recF")
            nc.vector.reciprocal(recF[:], sumsF[:])
            osb = wpool.tile([BL, A], f32, tag="osb")
            nc.vector.tensor_scalar_mul(osb[:], expF[:], recF[:])
            nc.sync.dma_start(out_dram[:, :], osb[:])

    nc.compile()
    return nc


def _host_prep(inputs):
    """Build the 8 per-core input maps from full inputs."""
    import ml_dtypes
    bf16 = ml_dtypes.bfloat16

    x = np.asarray(inputs["transformer_output"], np.float32)
    qmask = np.asarray(inputs["question_mask"])
    W_step = np.asarray(inputs["W_step"], np.float32)
    b_step = np.asarray(inputs["b_step"], np.float32).reshape(Dr, 1)
    w_att = np.asarray(inputs["w_att"], np.float32)
    rel_emb = np.asarray(inputs["rel_emb"], np.float32)
    ent_emb = np.asarray(inputs["ent_emb"], np.float32)
    ph = np.asarray(inputs["path_hidden"], np.float32)
    W1 = np.asarray(inputs["W1"], np.float32)
    b1 = np.asarray(inputs["b1"], np.float32).reshape(ACT, 1)
    W2 = np.asarray(inputs["W2"], np.float32)
    b2 = np.asarray(inputs["b2"], np.float32).reshape(1, ACT)
    r_space = np.asarray(inputs["r_space"], np.int32)
    e_space = np.asarray(inputs["e_space"], np.int32)
    action_mask = np.asarray(inputs["action_mask"], np.float32)

    def bf(a):
        return np.ascontiguousarray(a.astype(bf16))

    relwT = (rel_emb * w_att[None, :]).T                         # [Dr, R]
    w1a = W1[:H]
    w1b = W1[H:]
    c_rel = rel_emb @ b2[0, :Dr]
    c_ent = ent_emb @ b2[0, Dr:]

    w_step_b = bf(W_step)
    relwT_b = bf(relwT)
    w1a_b = bf(w1a)
    w1b_b = bf(w1b)
    w2_b = bf(W2)
    ones_b = bf(np.ones((1, 128), np.float32))

    in_maps = []
    for i in range(NCORES):
        b0, b1_ = i * BL, (i + 1) * BL
        xs = x[b0:b1_].reshape(BSL, Dw)
        mrow = np.where(qmask[b0:b1_].reshape(1, BSL), np.float32(NEG), np.float32(0.0))
        amask_add = np.where(action_mask[b0:b1_] > 0, np.float32(0.0), np.float32(NEG))
        amask_add = amask_add + c_rel[r_space[b0:b1_]] + c_ent[e_space[b0:b1_]]
        # host-side gather of action embedding rows, laid out [p, b*2+c, :]
        # where action a = c*128 + p
        ridx = r_space[b0:b1_].reshape(BL, 2, 128).transpose(2, 0, 1)   # [128, BL, 2]
        eidx = e_space[b0:b1_].reshape(BL, 2, 128).transpose(2, 0, 1)
        g_full = np.concatenate([rel_emb[ridx], ent_emb[eidx]], axis=-1)  # [128,BL,2,ACT]
        g_full = g_full.reshape(128, 2 * BL, ACT)
        # host-built one-hots: oh[r%128, b, r//128, a] = 1 for r = r_space[b,a]
        import ml_dtypes as _mld
        oh = np.zeros((128, BL, 4, A), _mld.bfloat16)
        rloc = r_space[b0:b1_]                                          # [BL, A]
        b_idx = np.repeat(np.arange(BL), A)
        a_idx = np.tile(np.arange(A), BL)
        rr = rloc.ravel()
        oh[rr % 128, b_idx, rr // 128, a_idx] = 1
        in_maps.append({
            "xT": bf(xs.T),
            "w_step": w_step_b,
            "b_step": b_step,
            "b_step_row": bf(b_step.reshape(1, Dr)),
            "relwT": relwT_b,
            "mask_row": bf(mrow),
            "oh": oh,
            "g_all": bf(g_full),
            "phT": bf(ph[b0:b1_].T),
            "w1a": w1a_b,
            "w1b": w1b_b,
            "b1": b1,
            "w2": w2_b,
            "amask": np.ascontiguousarray(amask_add),
            "ones_row": ones_b,
        })
    return in_maps


def kernel(**inputs):
    from concourse.bass_utils import run_bass_kernel_spmd

    if "nc" not in _CACHE:
        _CACHE["nc"] = _build()
    nc = _CACHE["nc"]
    in_maps = _host_prep(inputs)
    res = run_bass_kernel_spmd(nc, in_maps, list(range(NCORES)))
    return np.concatenate([res.results[i]["out"] for i in range(NCORES)], axis=0)


# revision 34
# speedup vs baseline: 1.6220x; 1.3121x over previous
"""Policy-network kernel for Trainium2 (Bass/Tile), SPMD over 8 NeuronCores.

Strategy: data-parallel over batch B=128 -> 16 batches per core; MLP weights
replicated; no collectives. Action embedding rows (rel_emb[r_space],
ent_emb[e_space]) are gathered host-side during input prep and shipped as one
bf16 tensor, so the device does no indirect DMA. Matmul operands are bf16
(same PE rate as float32r at N>=256, half the SBUF/DMA traffic). The per-(b,a)
relation gather of the attention output is fused on-chip via a one-hot matmul
whose one-hots are pre-scaled by the softmax normalizer. Final logits stay in
SBUF and are transposed on the PE for the action softmax (no DRAM round-trip).
"""

import numpy as np

# Problem dims (hardcoded per contract)
B, S, Dw, Dr, De, H, R, E, A = 128, 32, 300, 256, 256, 512, 512, 50000, 256
ACT = Dr + De          # 512
NCORES = 8
BL = B // NCORES       # 16 batches per core
BSL = BL * S           # 512 rows per core
NEG = -1e9

_CACHE = {}


def _build():
    import concourse.bass as bass
    import concourse.tile as tile
    from concourse import bacc, mybir

    f32 = mybir.dt.float32
    bf16 = mybir.dt.bfloat16
    ts = bass.ts

    nc = bacc.Bacc("TRN2", target_bir_lowering=False, debug=False)

    def din(name, shape, dt=bf16):
        return nc.dram_tensor(name, shape, dt, kind="ExternalInput").ap()

    xT = din("xT", [Dw, BSL])                    # transposed transformer_output shard
    w_step = din("w_step", [Dw, Dr])
    b_step = din("b_step", [Dr, 1], f32)
    b_step_row = din("b_step_row", [1, Dr])      # b_step as bf16 row for free-dim bias
    relwT = din("relwT", [Dr, R])                # (rel_emb * w_att).T
    mask_row = din("mask_row", [1, BSL])         # 0 / NEG per (b,s)
    oh_d = din("oh", [128, BL, 4, A])            # host-built one-hots of r_space
    g_all_d = din("g_all", [128, 2 * BL, ACT])   # host-gathered action embeddings
    phT = din("phT", [H, BL])                    # path_hidden.T shard
    w1a = din("w1a", [H, ACT])                   # W1[:H]
    w1b = din("w1b", [Dr, ACT])                  # W1[H:]
    b1 = din("b1", [ACT, 1], f32)
    w2 = din("w2", [ACT, ACT])
    amask = din("amask", [BL // 2, 2, A], f32)   # 0 / NEG with folded b2 terms, [b%8, b//8, a]
    ones_row = din("ones_row", [1, 128])

    out_dram = nc.dram_tensor("out", [BL, A], f32, kind="ExternalOutput").ap()

    KD = [(0, 128), (128, 256), (256, 300)]   # Dw K-tiles

    from concourse.masks import make_identity

    with tile.TileContext(nc) as tc:
        with (
            tc.tile_pool(name="const", bufs=1) as cpool,
            tc.tile_pool(name="work", bufs=2) as wpool,
            tc.tile_pool(name="perb", bufs=4) as bpool,
            tc.tile_pool(name="ps_big", bufs=2, space="PSUM") as ps_big,
            tc.tile_pool(name="ps_h2", bufs=2, space="PSUM") as ps_h2p,
            tc.tile_pool(name="ps_med", bufs=2, space="PSUM") as ps_med,
            tc.tile_pool(name="ps_ag", bufs=2, space="PSUM") as ps_agp,
        ):
            # ---- constants to SBUF ----
            ident_f32 = cpool.tile([128, 128], f32, tag="ident_f32")
            make_identity(nc, ident_f32[:])
            ones1 = cpool.tile([1, 128], bf16, tag="ones1")
            nc.sync.dma_start(ones1[:], ones_row[:])

            def load_const(tag, src, p0, p1, cols, dt=bf16):
                t = cpool.tile([p1 - p0, cols], dt, tag=tag)
                nc.sync.dma_start(t[:], src[p0:p1, :])
                return t
            # order: tensors gating the first computations go first
            w_step_sb = [load_const(f"ws{k}", w_step, a, b_, Dr) for k, (a, b_) in enumerate(KD)]
            xT_sb = [load_const(f"xt{k}", xT, a, b_, BSL) for k, (a, b_) in enumerate(KD)]
            b_step_sb = [load_const(f"bs{k}", b_step, k * 128, (k + 1) * 128, 1, f32) for k in range(2)]
            b_step_row_sb = cpool.tile([1, Dr], bf16, tag="b_step_row")
            nc.sync.dma_start(b_step_row_sb[:], b_step_row[:])
            relwT_sb = [load_const(f"rw{k}", relwT, k * 128, (k + 1) * 128, R) for k in range(2)]
            mask_sb = cpool.tile([1, BSL], bf16, tag="mask")
            nc.sync.dma_start(mask_sb[:], mask_row[:])
            # warm the ACT function table immediately (no data deps beyond b_step)
            act_warm = cpool.tile([128, 1], f32, tag="act_warm")
            nc.scalar.activation(act_warm[:], b_step_sb[0][:],
                                 bass.mybir.ActivationFunctionType.Exp, scale=0.0)
            oh_sb = cpool.tile([128, BL, 4, A], bf16, tag="oh_sb")
            for q in range(4):
                nc.sync.dma_start(oh_sb[:, ts(q, BL // 4), :, :],
                                  oh_d[:, ts(q, BL // 4), :, :])
            b1_sb = [load_const(f"b1{k}", b1, k * 128, (k + 1) * 128, 1, f32) for k in range(4)]
            w1a_sb = [load_const(f"w1a{k}", w1a, k * 128, (k + 1) * 128, ACT) for k in range(4)]
            phT_sb = [load_const(f"ph{k}", phT, k * 128, (k + 1) * 128, BL) for k in range(4)]
            w1b_sb = [load_const(f"w1b{k}", w1b, k * 128, (k + 1) * 128, ACT) for k in range(2)]
            w2_sb = [load_const(f"w2{k}", w2, k * 128, (k + 1) * 128, ACT) for k in range(4)]
            # host-gathered action embeddings: 4 column-chunk DMAs to spread queues
            g_all = cpool.tile([128, 2 * BL, ACT], bf16, tag="g_all")
            for q in range(4):
                nc.sync.dma_start(g_all[:, ts(q, BL // 2), :],
                                  g_all_d[:, ts(q, BL // 2), :])
            amask_sb = cpool.tile([BL // 2, 2, A], f32, tag="amask")
            nc.sync.dma_start(amask_sb[:], amask[:])

            # ---- saqT = tanh(W_step.T @ xT + b_step)  [2][128, BSL] bf16 ----
            saqT_sb = []
            for t in range(2):
                ps = ps_big.tile([128, BSL], f32, tag="big")
                for k in range(3):
                    nc.tensor.matmul(ps[:], w_step_sb[k][:, ts(t, 128)], xT_sb[k][:],
                                     start=(k == 0), stop=(k == 2))
                sb = cpool.tile([128, BSL], bf16, tag=f"saqT{t}")
                nc.scalar.activation(sb[:], ps[:], bass.mybir.ActivationFunctionType.Tanh,
                                     bias=b_step_sb[t][:])
                saqT_sb.append(sb)

            # ---- saq in natural row layout [128, 4, Dr]: chunk q holds rows
            # (b,s) = q*128+p, i.e. batches 4q..4q+3 (32 seq rows each) ----
            saq_nat = cpool.tile([128, 4, Dr], bf16, tag="saq_nat")
            for q in range(4):
                ps = ps_med.tile([128, Dr], f32, tag="med")
                for k in range(3):
                    nc.tensor.matmul(ps[:], xT_sb[k][:, ts(q, 128)], w_step_sb[k][:],
                                     start=(k == 0), stop=False)
                nc.tensor.matmul(ps[:], ones1[:], b_step_row_sb[:],
                                 start=False, stop=True)
                nc.scalar.activation(saq_nat[:, q, :], ps[:],
                                     bass.mybir.ActivationFunctionType.Tanh)

            # ---- scores -> normalized alpha [4][128, BL, S] bf16 (softmax over S;
            # exp on scalar, 1/sum applied as a broadcast mult on gpsimd) ----
            alpha_sb = []
            for rt in range(4):
                ps = ps_big.tile([128, BSL], f32, tag="big")
                for k in range(2):
                    nc.tensor.matmul(ps[:], relwT_sb[k][:, ts(rt, 128)], saqT_sb[k][:],
                                     start=(k == 0), stop=False)
                nc.tensor.matmul(ps[:], ones1[:], mask_sb[:], start=False, stop=True)
                al = wpool.tile([128, BL, S], bf16, tag="al_raw")
                nc.scalar.activation(al[:].rearrange("p b s -> p (b s)"), ps[:],
                                     bass.mybir.ActivationFunctionType.Exp)
                sums = wpool.tile([128, BL], f32, tag="sums")
                nc.vector.tensor_reduce(sums[:], al[:], axis=bass.mybir.AxisListType.X,
                                        op=bass.mybir.AluOpType.add)
                rec = wpool.tile([128, BL], f32, tag="rec")
                nc.vector.reciprocal(rec[:], sums[:])
                al_sc = cpool.tile([128, BL, S], bf16, tag=f"alpha{rt}")
                nc.gpsimd.tensor_tensor(al_sc[:], al[:],
                                        rec[:].unsqueeze(2).to_broadcast((128, BL, S)),
                                        op=bass.mybir.AluOpType.mult)
                alpha_sb.append(al_sc)

            # ---- ph contribution + fused h1 bias: biasT[t][:,b] = (ph@W1a)T + b1 ----
            biasT = cpool.tile([128, 4, BL], f32, tag="biasT")
            for t in range(4):
                ps = ps_med.tile([128, BL], f32, tag="med")
                for k in range(4):
                    nc.tensor.matmul(ps[:], w1a_sb[k][:, ts(t, 128)], phT_sb[k][:],
                                     start=(k == 0), stop=(k == 3))
                nc.vector.tensor_scalar_add(biasT[:, t, :], ps[:], b1_sb[t][:])

            # logits accumulators: lg_c[c][:, b] = logits for actions c*128..c*128+127
            lg_c = []
            for c in range(2):
                lg = cpool.tile([128, BL], f32, tag=f"lg{c}")
                lg_c.append(lg)

            HB = BL // 2

            def tail_half(h):
                # masked action softmax for batches h*HB..h*HB+HB-1, logits
                # transposed on the PE straight out of the accumulators
                sl = slice(h * HB, (h + 1) * HB)
                ps_t = ps_med.tile([HB, A], f32, tag="med")
                for c in range(2):
                    nc.tensor.transpose(ps_t[:, ts(c, 128)], lg_c[c][:, sl], ident_f32[:])
                lsb = wpool.tile([HB, A], f32, tag="lsb")
                nc.vector.tensor_add(lsb[:], ps_t[:], amask_sb[:, h, :])
                negmaxF = wpool.tile([HB, 1], f32, tag="negmaxF")
                nc.vector.tensor_reduce(negmaxF[:], lsb[:], axis=bass.mybir.AxisListType.X,
                                        op=bass.mybir.AluOpType.max, negate=True)
                expF = wpool.tile([HB, A], f32, tag="expF")
                sumsF = wpool.tile([HB, 1], f32, tag="sumsF")
                nc.scalar.activation(expF[:], lsb[:], bass.mybir.ActivationFunctionType.Exp,
                                     bias=negmaxF[:], accum_out=sumsF[:])
                recF = wpool.tile([HB, 1], f32, tag="recF")
                nc.vector.reciprocal(recF[:], sumsF[:])
                osb = wpool.tile([HB, A], f32, tag="osb")
                nc.vector.tensor_scalar_mul(osb[:], expF[:], recF[:])
                nc.sync.dma_start(out_dram[sl, :], osb[:])

            # ---- per-batch pipeline, stage-major over groups of 4 ----
            alpha_gTs, raq_gTs, h1Ts = {}, {}, {}
            for g in range(BL // 4):
                bs = range(4 * g, 4 * g + 4)
                for b in bs:
                    off = (b % 4) * S
                    ps_ag = ps_agp.tile([128, A], f32, tag="ag")
                    for rt in range(4):
                        nc.tensor.matmul(ps_ag[off:off + S, :], alpha_sb[rt][:, b, :],
                                         oh_sb[:, b, rt, :],
                                         start=(rt == 0), stop=(rt == 3),
                                         tile_position=(0, off))
                    alpha_gT = bpool.tile([128, A], bf16, tag="alpha_gT", bufs=6)
                    nc.vector.tensor_copy(alpha_gT[off:off + S, :], ps_ag[off:off + S, :])
                    alpha_gTs[b] = alpha_gT
                for b in bs:
                    off = (b % 4) * S
                    raq_gT = bpool.tile([128, 2, A], bf16, tag="raq_gT", bufs=5)
                    ps_rq = ps_big.tile([128, 2 * A], f32, tag="big")
                    sq = saq_nat[off:off + S, b // 4, :]
                    for dt_ in range(2):
                        nc.tensor.matmul(ps_rq[:, ts(dt_, A)], sq[:, ts(dt_, 128)],
                                         alpha_gTs[b][off:off + S, :], start=True, stop=True,
                                         tile_position=(off, 0))
                    nc.scalar.copy(raq_gT[:], ps_rq[:].rearrange("p (d a) -> p d a", a=A))
                    raq_gTs[b] = raq_gT
                for b in bs:
                    h1T = bpool.tile([128, 4, A], bf16, tag="h1T", bufs=5)
                    for t in range(4):
                        ps_h1 = ps_med.tile([128, A], f32, tag="med")
                        for k in range(2):
                            nc.tensor.matmul(ps_h1[:], w1b_sb[k][:, ts(t, 128)],
                                             raq_gTs[b][:, k, :], start=(k == 0), stop=(k == 1))
                        nc.scalar.activation(h1T[:, t, :], ps_h1[:],
                                             bass.mybir.ActivationFunctionType.Relu,
                                             bias=biasT[:, t, b:b + 1])
                    h1Ts[b] = h1T
                for b in bs:
                    for c in range(2):
                        ps_h2 = ps_h2p.tile([128, ACT], f32, tag="h2")
                        for k in range(4):
                            nc.tensor.matmul(ps_h2[:], h1Ts[b][:, k, ts(c, 128)], w2_sb[k][:],
                                             start=(k == 0), stop=(k == 3))
                        scratch = bpool.tile([128, ACT], f32, tag="scratch", bufs=3)
                        nc.vector.tensor_mul(scratch[:], ps_h2[:], g_all[:, b * 2 + c, :])
                        nc.vector.tensor_reduce(lg_c[c][:, b:b + 1], scratch[:],
                                                axis=bass.mybir.AxisListType.X,
                                                op=bass.mybir.AluOpType.add)
                if g == BL // 8 - 1:
                    tail_half(0)
            tail_half(1)

    nc.compile()
    return nc


def _host_prep(inputs):
    """Build the 8 per-core input maps from full inputs."""
    import ml_dtypes
    bf16 = ml_dtypes.bfloat16

    x = np.asarray(inputs["transformer_output"], np.float32)
    qmask = np.asarray(inputs["question_mask"])
    W_step = np.asarray(inputs["W_step"], np.float32)
    b_step = np.asarray(inputs["b_step"], np.float32).reshape(Dr, 1)
    w_att = np.asarray(inputs["w_att"], np.float32)
    rel_emb = np.asarray(inputs["rel_emb"], np.float32)
    ent_emb = np.asarray(inputs["ent_emb"], np.float32)
    ph = np.asarray(inputs["path_hidden"], np.float32)
    W1 = np.asarray(inputs["W1"], np.float32)
    b1 = np.asarray(inputs["b1"], np.float32).reshape(ACT, 1)
    W2 = np.asarray(inputs["W2"], np.float32)
    b2 = np.asarray(inputs["b2"], np.float32).reshape(1, ACT)
    r_space = np.asarray(inputs["r_space"], np.int32)
    e_space = np.asarray(inputs["e_space"], np.int32)
    action_mask = np.asarray(inputs["action_mask"], np.float32)

    def bf(a):
        return np.ascontiguousarray(a.astype(bf16))

    relwT = (rel_emb * w_att[None, :]).T                         # [Dr, R]
    w1a = W1[:H]
    w1b = W1[H:]
    c_rel = rel_emb @ b2[0, :Dr]
    c_ent = ent_emb @ b2[0, Dr:]

    w_step_b = bf(W_step)
    relwT_b = bf(relwT)
    w1a_b = bf(w1a)
    w1b_b = bf(w1b)
    w2_b = bf(W2)
    ones_b = bf(np.ones((1, 128), np.float32))

    in_maps = []
    for i in range(NCORES):
        b0, b1_ = i * BL, (i + 1) * BL
        xs = x[b0:b1_].reshape(BSL, Dw)
        mrow = np.where(qmask[b0:b1_].reshape(1, BSL), np.float32(NEG), np.float32(0.0))
        amask_add = np.where(action_mask[b0:b1_] > 0, np.float32(0.0), np.float32(NEG))
        amask_add = amask_add + c_rel[r_space[b0:b1_]] + c_ent[e_space[b0:b1_]]
        # host-side gather of action embedding rows, laid out [p, b*2+c, :]
        # where action a = c*128 + p
        ridx = r_space[b0:b1_].reshape(BL, 2, 128).transpose(2, 0, 1)   # [128, BL, 2]
        eidx = e_space[b0:b1_].reshape(BL, 2, 128).transpose(2, 0, 1)
        g_full = np.concatenate([rel_emb[ridx], ent_emb[eidx]], axis=-1)  # [128,BL,2,ACT]
        g_full = g_full.reshape(128, 2 * BL, ACT)
        # host-built one-hots: oh[r%128, b, r//128, a] = 1 for r = r_space[b,a]
        import ml_dtypes as _mld
        oh = np.zeros((128, BL, 4, A), _mld.bfloat16)
        rloc = r_space[b0:b1_]                                          # [BL, A]
        b_idx = np.repeat(np.arange(BL), A)
        a_idx = np.tile(np.arange(A), BL)
        rr = rloc.ravel()
        oh[rr % 128, b_idx, rr // 128, a_idx] = 1
        in_maps.append({
            "xT": bf(xs.T),
            "w_step": w_step_b,
            "b_step": b_step,
            "b_step_row": bf(b_step.reshape(1, Dr)),
            "relwT": relwT_b,
            "mask_row": bf(mrow),
            "oh": oh,
            "g_all": bf(g_full),
            "phT": bf(ph[b0:b1_].T),
            "w1a": w1a_b,
            "w1b": w1b_b,
            "b1": b1,
            "w2": w2_b,
            "amask": np.ascontiguousarray(
                amask_add.reshape(2, BL // 2, A).transpose(1, 0, 2)),
            "ones_row": ones_b,
        })
    return in_maps


def kernel(**inputs):
    from concourse.bass_utils import run_bass_kernel_spmd

    if "nc" not in _CACHE:
        _CACHE["nc"] = _build()
    nc = _CACHE["nc"]
    in_maps = _host_prep(inputs)
    res = run_bass_kernel_spmd(nc, in_maps, list(range(NCORES)))
    return np.concatenate([res.results[i]["out"] for i in range(NCORES)], axis=0)
